# revision 1
# baseline (speedup 1.0000x reference)
# Depthwise causal conv1d (B=8, T=4096, C=1024, K=4, dilation=1) on 8 TRN2
# NeuronCores.
#
# Math: y[b, t, c] = sum_{j=0..3} weight[c, 3-j] * x[b, t-j, c]   (x[t<0] = 0)
#
# Strategy:
#   - Shard batch: core b handles x[b] (one full (T, C) slice).
#   - Host transposes each shard to (C, T) so the time axis is contiguous in
#     DRAM and lands on the SBUF free dimension; channels land on partitions.
#   - On-chip: for each 128-channel block, one [128, T+3] SBUF tile (3-col
#     zero halo at the left edge).  The 4 taps are applied by the TensorEngine
#     as 4 accumulating matmuls with a per-block *diagonal* weight matrix
#     lhsT = diag(w[cblock, 3-j]) against time-shifted rhs slices; PSUM does
#     the 4-tap accumulation for free.  fp32r keeps the PE at 1 cycle/row.
#   - DVE/ACT alternate on PSUM->SBUF copies; HWDGE DMAs move 2MB rows.
#   - Host transposes results back and stacks to (B, T, C).

import numpy as np

B, T, C, K = 8, 4096, 1024, 4
N_CORES = 8
P = 128  # SBUF partitions
NSUB = 512  # matmul free-dim (one fp32 PSUM bank)
HALO = 4  # leading zero columns (causal left pad), shipped from host

_CACHE = {}


def _build_nc(t_len=T, n_ch=C, mode="f32r"):
    import concourse.mybir as mybir
    import concourse.tile as tile
    from concourse import bacc
    from concourse.masks import make_identity

    f32 = mybir.dt.float32
    if mode == "f32r":
        cdt = mybir.dt.float32r
    elif mode == "bf16":
        cdt = mybir.dt.bfloat16
    else:
        cdt = f32
    ncb = n_ch // P  # channel blocks
    nsub = t_len // NSUB  # time sub-blocks per channel block

    # Bacc (not raw Bass): its compile() pass legalizes multi-wait sync into
    # event-semaphore instructions (TRN2 allows 1 wait per instruction).
    nc = bacc.Bacc(None)
    # x is declared with the compute dtype; for f32r this is a bit-identical
    # view of f32, for bf16 the (SWDGE) DMA casts inline.
    x_dt = cdt if mode == "f32r" else f32
    x = nc.declare_dram_parameter("x", [n_ch, t_len + HALO], x_dt, isOutput=False)
    # w_sb[p, cb*K + jj] = weight[cb*128 + p, jj]; diagonal lhsT blocks are
    # built on-chip (identity x per-partition scalar) to avoid a 2MB DMA.
    w = nc.declare_dram_parameter("w", [P, ncb * K], f32, isOutput=False)
    y = nc.declare_dram_parameter("y", [n_ch, t_len], f32, isOutput=True)

    # Each 128-channel block's time axis is processed as two half-rows of
    # t_len/2, each its own SBUF tile, so loads/stores move 1MB grains and
    # stores can start after half the block's PSUM copies.
    half = t_len // 2
    hsub = half // NSUB  # sub-blocks per half

    with tile.TileContext(nc) as tc:
        with (
            tc.tile_pool(name="const", bufs=1) as cpool,
            tc.tile_pool(name="xin", bufs=6) as xpool,
            tc.tile_pool(name="yout", bufs=4) as ypool,
            tc.tile_pool(name="ps", bufs=8, space="PSUM") as pspool,
        ):
            w_sb = cpool.tile([P, ncb * K], f32)
            nc.sync.dma_start(out=w_sb[:, :], in_=w[:, :])
            ident = cpool.tile([P, P], f32)
            make_identity(nc, ident)
            # wdiag[(cb, j)] holds diag(weight[cb*128 + p, K-1-j]).  One
            # tile per block: Tile tracks deps per tile, so the first
            # matmul only waits for its own diag, not all 32 builder ops.
            wdiag = {}
            for cb in range(ncb):
                for j in range(K):
                    col = cb * K + (K - 1 - j)
                    wd = cpool.tile([P, P], cdt, tag=f"wd_{cb}_{j}")
                    nc.vector.tensor_scalar_mul(
                        out=wd[:, :],
                        in0=ident[:, :],
                        scalar1=w_sb[:, col : col + 1],
                    )
                    wdiag[(cb, j)] = wd

            xdma = nc.gpsimd if mode == "bf16" else nc.sync
            for cb in range(ncb):
                rows = slice(cb * P, (cb + 1) * P)
                for h in range(2):
                    # half h covers t in [h*half, (h+1)*half); each x tile
                    # has HALO extra leading cols (zero pad for h=0, shipped
                    # by the host; overlap re-load of the previous 4 cols
                    # otherwise) so fp32r matmuls stay at N=512 any offset.
                    # The very first half-block is loaded as four 512-col
                    # piece-tiles so the PE starts after ~260KB, not 1MB.
                    first = cb == 0 and h == 0
                    if first:
                        xts = []
                        for m in range(hsub):
                            xp = xpool.tile([P, NSUB + HALO], cdt, tag="xhead")
                            xdma.dma_start(
                                out=xp[:, :],
                                in_=x[rows, NSUB * m : NSUB * (m + 1) + HALO],
                            )
                            xts.append(xp)
                    else:
                        xt = xpool.tile([P, half + HALO], cdt)
                        xdma.dma_start(
                            out=xt[:, :],
                            in_=x[rows, h * half : (h + 1) * half + HALO],
                        )
                    yt = ypool.tile([P, half], f32)
                    for m in range(hsub):
                        ps = pspool.tile([P, NSUB], f32)
                        for j in range(K):
                            # y[:, t] += diag(w[:, K-1-j]) @ x[:, t - j]
                            lhsT = wdiag[(cb, j)][:, :]
                            if first:
                                rhs = xts[m][:, HALO - j : HALO - j + NSUB]
                            else:
                                off = NSUB * m + HALO - j
                                rhs = xt[:, off : off + NSUB]
                            nc.tensor.matmul(
                                ps[:, :], lhsT, rhs,
                                start=(j == 0), stop=(j == K - 1),
                            )
                        dst = yt[:, NSUB * m : NSUB * (m + 1)]
                        if m % 2 == 0:
                            nc.vector.tensor_copy(dst, ps[:, :])
                        else:
                            nc.scalar.copy(dst, ps[:, :])
                    # Stores go out on the ACT HWDGE ring (nc.scalar) so they
                    # don't head-of-line-block the x loads on the SP ring.
                    nc.scalar.dma_start(
                        out=y[rows, h * half : (h + 1) * half], in_=yt[:, :]
                    )
    return nc


MODE = "f32r"  # compute dtype for the PE: "f32r" (2e-4 err) or "bf16" (faster)


def _get_nc():
    if "nc" not in _CACHE:
        nc = _build_nc(mode=MODE)
        # Bacc.finalize() runs compile(): moves matmul waits to ldweights,
        # splits multi-wait sync into event-sem instructions, allocates regs.
        nc.finalize()
        _CACHE["nc"] = nc
    return _CACHE["nc"]


def _pack_weight(weight):
    # w_sb[p, cb*K + jj] = weight[cb*P + p, jj]
    w = np.asarray(weight, dtype=np.float32)
    ncb = C // P
    return np.ascontiguousarray(
        w.reshape(ncb, P, K).transpose(1, 0, 2).reshape(P, ncb * K)
    )


LAST_RESULT = None


def kernel(x, weight):
    global LAST_RESULT
    from concourse.bass_utils import run_bass_kernel_spmd

    x = np.asarray(x, dtype=np.float32)
    w_sb = _pack_weight(weight)
    nc = _get_nc()

    in_maps = []
    for b in range(N_CORES):
        xt = np.zeros((C, T + HALO), dtype=np.float32)
        xt[:, HALO:] = x[b].T
        in_maps.append({"x": xt, "w": w_sb})
    res = run_bass_kernel_spmd(nc, in_maps, list(range(N_CORES)))
    LAST_RESULT = res

    y = np.empty((B, T, C), dtype=np.float32)
    for b in range(N_CORES):
        y[b] = res.results[b]["y"].T
    return y



# revision 2
# speedup vs baseline: 1.4361x; 1.4361x over previous
# Depthwise causal conv1d (B=8, T=4096, C=1024, K=4, dilation=1) on 8 TRN2
# NeuronCores.
#
# Math: y[b, t, c] = sum_{j=0..3} weight[c, 3-j] * x[b, t-j, c]   (x[t<0] = 0)
#
# Strategy (v2 — fp16 I/O, PE+DVE compute split):
#   - Shard batch: core b handles x[b] (one full (T, C) slice).
#   - Host transposes each shard to (C, T) and casts to fp16, so device DMA
#     traffic is halved vs fp32: 8.2MB in + 8.2MB out per core.  At the
#     360 GB/s per-core DMA roofline (shared by loads+stores) that's ~47us,
#     vs ~94us for the fp32 baseline.  fp16 keeps 11 sig bits: worst-case
#     abs err ~1e-2 against an output scale of ~3.2 (gate is 2e-2 rel).
#   - On-chip, per 128-channel block: one [128, T+4] fp16 tile (4-col zero
#     halo at the left edge for the causal pad).  The 8 512-col subtiles are
#     split between engines so compute hides under DMA:
#       * subtiles 0..4 -> TensorE: 4 accumulating matmuls with per-block
#         diagonal fp16 weights (PSUM does the tap sum); ACT copies
#         PSUM->SBUF with an inline fp32->fp16 cast.
#       * subtiles 5..7 -> DVE as one 1536-col slab: 4 tensor_scalar mults
#         (4x_2p mode: 4 elem/cycle/lane for 2-byte SBUF operands) + 3
#         tensor_tensor adds (2x_1p: 2 elem/cycle/lane).
#   - Loads ride the SP HWDGE ring, stores the ACT ring.
#   - Host casts fp16 results back to fp32 and re-transposes.

import numpy as np

B, T, C, K = 8, 4096, 1024, 4
N_CORES = 8
P = 128  # SBUF partitions
NSUB = 512  # PE subtile width (one fp32 PSUM bank)
HALO = 4  # leading zero columns (causal left pad), shipped from host
PE_SUB = 5  # subtiles 0..PE_SUB-1 on TensorE, the rest on DVE

_CACHE = {}


def _build_nc():
    import concourse.mybir as mybir
    import concourse.tile as tile
    from concourse import bacc
    from concourse.masks import make_identity

    f32 = mybir.dt.float32
    f16 = mybir.dt.float16
    add = mybir.AluOpType.add
    ncb = C // P  # channel blocks per core

    nc = bacc.Bacc(None)
    x = nc.declare_dram_parameter("x", [C, T + HALO], f16, isOutput=False)
    # w_sb[p, cb*K + jj] = weight[cb*128 + p, jj]
    w = nc.declare_dram_parameter("w", [P, ncb * K], f32, isOutput=False)
    y = nc.declare_dram_parameter("y", [C, T], f16, isOutput=True)

    s0 = PE_SUB * NSUB  # first DVE column
    L = T - s0  # DVE slab width

    with tile.TileContext(nc) as tc:
        with (
            tc.tile_pool(name="const", bufs=1) as cpool,
            tc.tile_pool(name="xin", bufs=3) as xpool,
            tc.tile_pool(name="yout", bufs=3) as ypool,
            tc.tile_pool(name="tmp", bufs=2) as tpool,
            tc.tile_pool(name="ps", bufs=8, space="PSUM") as pspool,
        ):
            w_sb = cpool.tile([P, ncb * K], f32)
            nc.sync.dma_start(out=w_sb[:, :], in_=w[:, :])
            ident = cpool.tile([P, P], f16)
            make_identity(nc, ident)
            # wdiag[(cb, j)] holds diag(weight[cb*128 + p, K-1-j]) in fp16.
            wdiag = {}
            for cb in range(ncb):
                for j in range(K):
                    col = cb * K + (K - 1 - j)
                    wd = cpool.tile([P, P], f16, tag=f"wd_{cb}_{j}")
                    nc.vector.tensor_scalar_mul(
                        out=wd[:, :],
                        in0=ident[:, :],
                        scalar1=w_sb[:, col : col + 1],
                    )
                    wdiag[(cb, j)] = wd

            for cb in range(ncb):
                rows = slice(cb * P, (cb + 1) * P)
                xt = xpool.tile([P, T + HALO], f16)
                nc.sync.dma_start(out=xt[:, :], in_=x[rows, :])
                yt = ypool.tile([P, T], f16)

                # --- TensorE subtiles ---
                for m in range(PE_SUB):
                    ps = pspool.tile([P, NSUB], f32)
                    for j in range(K):
                        off = HALO + NSUB * m - j
                        nc.tensor.matmul(
                            ps[:, :],
                            wdiag[(cb, j)][:, :],
                            xt[:, off : off + NSUB],
                            start=(j == 0),
                            stop=(j == K - 1),
                        )
                    nc.scalar.copy(yt[:, NSUB * m : NSUB * (m + 1)], ps[:, :])

                # --- DVE slab: y[:, s0:] = sum_j w_j * x[:, s0-j : s0-j+L] ---
                def xoff(j):
                    off = HALO + s0 - j
                    return xt[:, off : off + L]

                def wcol(j):
                    col = cb * K + (K - 1 - j)
                    return w_sb[:, col : col + 1]

                a = tpool.tile([P, L], f16, tag="a")
                bb = tpool.tile([P, L], f16, tag="b")
                cc = tpool.tile([P, L], f16, tag="c")
                dd = tpool.tile([P, L], f16, tag="d")
                nc.vector.tensor_scalar_mul(out=a[:, :], in0=xoff(0), scalar1=wcol(0))
                nc.vector.tensor_scalar_mul(out=bb[:, :], in0=xoff(1), scalar1=wcol(1))
                nc.vector.tensor_tensor(
                    out=a[:, :], in0=a[:, :], in1=bb[:, :], op=add
                )
                nc.vector.tensor_scalar_mul(out=cc[:, :], in0=xoff(2), scalar1=wcol(2))
                nc.vector.tensor_scalar_mul(out=dd[:, :], in0=xoff(3), scalar1=wcol(3))
                nc.vector.tensor_tensor(
                    out=cc[:, :], in0=cc[:, :], in1=dd[:, :], op=add
                )
                nc.vector.tensor_tensor(
                    out=yt[:, s0:], in0=a[:, :], in1=cc[:, :], op=add
                )

                nc.scalar.dma_start(out=y[rows, :], in_=yt[:, :])
    return nc


def _get_nc():
    if "nc" not in _CACHE:
        nc = _build_nc()
        nc.finalize()
        _CACHE["nc"] = nc
    return _CACHE["nc"]


def _pack_weight(weight):
    # w_sb[p, cb*K + jj] = weight[cb*P + p, jj]
    w = np.asarray(weight, dtype=np.float32)
    ncb = C // P
    return np.ascontiguousarray(
        w.reshape(ncb, P, K).transpose(1, 0, 2).reshape(P, ncb * K)
    )


def _prep_inputs(x, weight):
    x = np.asarray(x)
    w_sb = _pack_weight(weight)
    in_maps = []
    for b in range(N_CORES):
        xt = np.zeros((C, T + HALO), dtype=np.float16)
        xt[:, HALO:] = x[b].T
        in_maps.append({"x": xt, "w": w_sb})
    return in_maps


def _collect_output(res):
    y = np.empty((B, T, C), dtype=np.float32)
    for b in range(N_CORES):
        y[b] = res.results[b]["y"].T.astype(np.float32)
    return y


LAST_RESULT = None


def kernel(x, weight):
    global LAST_RESULT
    from concourse.bass_utils import run_bass_kernel_spmd

    in_maps = _prep_inputs(x, weight)
    nc = _get_nc()
    res = run_bass_kernel_spmd(nc, in_maps, list(range(N_CORES)))
    LAST_RESULT = res
    return _collect_output(res)
